# revision 6
# baseline (speedup 1.0000x reference)
"""Bayesian linear layer (reparameterized sampling) on 8 Trainium2 NeuronCores.

Computes y = x @ (mu + softplus(rho) * eps_w)^T + (bias_mu + softplus(bias_rho) * eps_b)
with x [8192, 4096], weights [4096, 4096].

Strategy: column-parallel tensor parallelism. Each of the 8 cores owns a
512-wide slice of out_features: it materializes its weight slice
w_c = mu_c + softplus(rho_c) * eps_c on-chip (ACT softplus + DVE mul/add,
downcast to bf16), then computes y_c^T = w_c @ x^T with the TensorEngine
(bf16 matmul, fp32 PSUM accumulation), fusing the bias add into the
PSUM->SBUF copy on the scalar engine. x is replicated to all cores as
bf16 in [in_features, tokens] layout so the contraction dim lands on
partitions with no on-chip transposes. Outputs stay sharded ([512, 8192]
per core) and are concatenated/transposed on the host.
"""

import sys

for _p in ("/opt/trn_rl_repo",):
    if _p not in sys.path:
        sys.path.insert(0, _p)

import numpy as np
import ml_dtypes

IN_F = 4096
OUT_F = 4096
TOKENS = 8192
NCORES = 8
O_SH = OUT_F // NCORES  # 512 out-features per core

P = 128
NF = 512  # matmul free dim (one PSUM bank of fp32)


def build_nc(in_f=IN_F, o_sh=O_SH, tokens=TOKENS):
    """Build the per-core Bass graph. All cores run the same graph (SPMD)."""
    import concourse.bass as bass  # noqa: F401
    import concourse.mybir as mybir
    from concourse import bacc, tile

    f32 = mybir.dt.float32
    bf16 = mybir.dt.bfloat16
    KO = in_f // P        # k tiles of 128
    MS = o_sh // P        # psum-partition (out-feature) subtiles
    NT = tokens // NF     # token chunks
    KG = 2                # k tiles per weight-precompute chunk
    EXP = mybir.ActivationFunctionType.Exp
    LN = mybir.ActivationFunctionType.Ln
    IDENT = mybir.ActivationFunctionType.Identity

    nc = bacc.Bacc(None, target_bir_lowering=False)

    xT = nc.declare_dram_parameter("xT", [in_f, tokens], bf16, False)
    muT = nc.declare_dram_parameter("muT", [in_f, o_sh], f32, False)
    rhoT = nc.declare_dram_parameter("rhoT", [in_f, o_sh], f32, False)
    epsT = nc.declare_dram_parameter("epsT", [in_f, o_sh], f32, False)
    bmu = nc.declare_dram_parameter("bmu", [P, MS], f32, False)
    brho = nc.declare_dram_parameter("brho", [P, MS], f32, False)
    beps = nc.declare_dram_parameter("beps", [P, MS], f32, False)
    out = nc.declare_dram_parameter("out", [o_sh, tokens], f32, True)

    # Partition-tiled views: row index r = ko*128 + p
    xT3 = xT[:].rearrange("(ko p) t -> p ko t", p=P)
    muT3 = muT[:].rearrange("(ko p) m -> p ko m", p=P)
    rhoT3 = rhoT[:].rearrange("(ko p) m -> p ko m", p=P)
    epsT3 = epsT[:].rearrange("(ko p) m -> p ko m", p=P)
    out3 = out[:].rearrange("(ms p) t -> p ms t", p=P)

    with tile.TileContext(nc) as tc:
        with (
            tc.tile_pool(name="wpool", bufs=1) as wpool,
            tc.tile_pool(name="bias", bufs=1) as bias_pool,
            tc.tile_pool(name="xpool", bufs=2) as xpool,
            tc.tile_pool(name="opool", bufs=6) as opool,
            tc.tile_pool(name="psum", bufs=8, space="PSUM") as psum_pool,
        ):
            # ---- bias: b = bias_mu + softplus(bias_rho) * eps_b  ([128, MS])
            bmu_t = bias_pool.tile([P, MS], f32, tag="bmu")
            nc.sync.dma_start(bmu_t[:], bmu[:])
            brho_t = bias_pool.tile([P, MS], f32, tag="brho")
            nc.sync.dma_start(brho_t[:], brho[:])
            beps_t = bias_pool.tile([P, MS], f32, tag="beps")
            nc.sync.dma_start(beps_t[:], beps[:])
            # softplus(v) = ln(exp(v) + 1) — Softplus has no ACT table on gen3
            b_sp = bias_pool.tile([P, MS], f32, tag="bsp")
            nc.scalar.activation(b_sp[:], brho_t[:], EXP)
            nc.scalar.activation(b_sp[:], b_sp[:], LN, bias=1.0)
            b_sb = bias_pool.tile([P, MS], f32, tag="bsb")
            nc.vector.tensor_mul(b_sb[:], b_sp[:], beps_t[:])
            nc.vector.tensor_add(b_sb[:], b_sb[:], bmu_t[:])

            # ---- weights: wT = mu + softplus(rho) * eps, downcast to bf16
            # wT layout [128, KO, o_sh]: lhsT tiles for the matmul.
            wT = wpool.tile([P, KO, o_sh], bf16, tag="wT")
            with tc.tile_pool(name="wtmp", bufs=2) as wtmp:
                for kg in range(KO // KG):
                    sl = slice(kg * KG, (kg + 1) * KG)
                    rho_t = wtmp.tile([P, KG, o_sh], f32, tag="rho")
                    nc.sync.dma_start(rho_t[:], rhoT3[:, sl, :])
                    eps_t = wtmp.tile([P, KG, o_sh], f32, tag="eps")
                    nc.sync.dma_start(eps_t[:], epsT3[:, sl, :])
                    mu_t = wtmp.tile([P, KG, o_sh], f32, tag="mu")
                    nc.sync.dma_start(mu_t[:], muT3[:, sl, :])
                    sp_t = wtmp.tile([P, KG, o_sh], f32, tag="sp")
                    nc.scalar.activation(sp_t[:], rho_t[:], EXP)
                    nc.scalar.activation(sp_t[:], sp_t[:], LN, bias=1.0)
                    pr_t = wtmp.tile([P, KG, o_sh], f32, tag="pr")
                    nc.vector.tensor_mul(pr_t[:], sp_t[:], eps_t[:])
                    nc.vector.tensor_add(wT[:, sl, :], pr_t[:], mu_t[:])

            # ---- main loop: y^T[o, t] += w[o, i] * x[t, i]
            for n in range(NT):
                tsl = slice(n * NF, (n + 1) * NF)
                x_t = xpool.tile([P, KO, NF], bf16, tag="x")
                nc.sync.dma_start(x_t[:], xT3[:, :, tsl])
                for ms in range(MS):
                    ps = psum_pool.tile([P, NF], f32, tag="ps")
                    for ko in range(KO):
                        nc.tensor.matmul(
                            ps[:],
                            wT[:, ko : ko + 1, ms * P : (ms + 1) * P],
                            x_t[:, ko : ko + 1, :],
                            start=(ko == 0),
                            stop=(ko == KO - 1),
                        )
                    o_t = opool.tile([P, NF], f32, tag="o")
                    nc.scalar.activation(
                        o_t[:], ps[:], IDENT, bias=b_sb[:, ms : ms + 1], scale=1.0
                    )
                    nc.sync.dma_start(out3[:, ms, tsl], o_t[:])

    nc.compile()
    return nc


def shard_inputs(x, weight_mu, weight_rho, bias_mu, bias_rho, eps_w, eps_b,
                 in_f=IN_F, o_sh=O_SH, tokens=TOKENS, ncores=NCORES):
    """Host-side layout + sharding: transpose to [in, out] / [in, tokens]."""
    bf16 = ml_dtypes.bfloat16
    MS = o_sh // P
    xT_bf = np.ascontiguousarray(np.asarray(x, dtype=np.float32).astype(bf16).T)
    in_maps = []
    for c in range(ncores):
        sl = slice(c * o_sh, (c + 1) * o_sh)
        in_maps.append({
            "xT": xT_bf,
            "muT": np.ascontiguousarray(np.asarray(weight_mu)[sl, :].T),
            "rhoT": np.ascontiguousarray(np.asarray(weight_rho)[sl, :].T),
            "epsT": np.ascontiguousarray(np.asarray(eps_w)[sl, :].T),
            "bmu": np.ascontiguousarray(np.asarray(bias_mu)[sl].reshape(MS, P).T),
            "brho": np.ascontiguousarray(np.asarray(bias_rho)[sl].reshape(MS, P).T),
            "beps": np.ascontiguousarray(np.asarray(eps_b)[sl].reshape(MS, P).T),
        })
    return in_maps


_NC_CACHE = {}


def _get_nc():
    if "nc" not in _NC_CACHE:
        _NC_CACHE["nc"] = build_nc()
    return _NC_CACHE["nc"]


def kernel(x, weight_mu, weight_rho, bias_mu, bias_rho, eps_w, eps_b):
    from concourse import bass_utils

    nc = _get_nc()
    in_maps = shard_inputs(x, weight_mu, weight_rho, bias_mu, bias_rho, eps_w, eps_b)
    res = bass_utils.run_bass_kernel_spmd(nc, in_maps, core_ids=list(range(NCORES)))
    yT = np.concatenate([res.results[c]["out"] for c in range(NCORES)], axis=0)
    return np.ascontiguousarray(yT.T)


# revision 9
# speedup vs baseline: 30.7158x; 30.7158x over previous
"""Bayesian linear layer (reparameterized sampling) on 8 Trainium2 NeuronCores.

Computes y = x @ (mu + softplus(rho) * eps_w)^T + (bias_mu + softplus(bias_rho) * eps_b)
with x [8192, 4096], weights [4096, 4096].

Strategy: column-parallel tensor parallelism. Each of the 8 cores owns a
512-wide slice of out_features: it materializes its weight slice
w_c = mu_c + softplus(rho_c) * eps_c on-chip (ACT softplus + DVE mul/add,
downcast to bf16), then computes y_c^T = w_c @ x^T with the TensorEngine
(bf16 matmul, fp32 PSUM accumulation), fusing the bias add into the
PSUM->SBUF copy on the scalar engine. x is replicated to all cores as
bf16 in [in_features, tokens] layout so the contraction dim lands on
partitions with no on-chip transposes. Outputs stay sharded ([512, 8192]
per core) and are concatenated/transposed on the host.
"""

import sys

for _p in ("/opt/trn_rl_repo",):
    if _p not in sys.path:
        sys.path.insert(0, _p)

import numpy as np
import ml_dtypes

IN_F = 4096
OUT_F = 4096
TOKENS = 8192
NCORES = 8
O_SH = OUT_F // NCORES  # 512 out-features per core

P = 128
NF = 512  # matmul free dim (one PSUM bank of fp32)


def build_nc(in_f=IN_F, o_sh=O_SH, tokens=TOKENS, wrepeat=1, mrepeat=1):
    """Build the per-core Bass graph. All cores run the same graph (SPMD).

    wrepeat/mrepeat repeat the weight-precompute / main loop for
    slope-based benchmarking (identical redundant work); production is 1/1.
    """
    import concourse.bass as bass  # noqa: F401
    import concourse.mybir as mybir
    from concourse import bacc, tile

    f32 = mybir.dt.float32
    bf16 = mybir.dt.bfloat16
    KO = in_f // P        # k tiles of 128
    MS = o_sh // P        # psum-partition (out-feature) subtiles
    NT = tokens // NF     # token chunks
    KG = 2                # k tiles per weight-precompute chunk
    EXP = mybir.ActivationFunctionType.Exp
    LN = mybir.ActivationFunctionType.Ln
    IDENT = mybir.ActivationFunctionType.Identity

    nc = bacc.Bacc(None, target_bir_lowering=False)

    xT = nc.declare_dram_parameter("xT", [in_f, tokens], bf16, False)
    muT = nc.declare_dram_parameter("muT", [in_f, o_sh], f32, False)
    rhoT = nc.declare_dram_parameter("rhoT", [in_f, o_sh], f32, False)
    epsT = nc.declare_dram_parameter("epsT", [in_f, o_sh], f32, False)
    bmu = nc.declare_dram_parameter("bmu", [P, MS], f32, False)
    brho = nc.declare_dram_parameter("brho", [P, MS], f32, False)
    beps = nc.declare_dram_parameter("beps", [P, MS], f32, False)
    out = nc.declare_dram_parameter("out", [o_sh, tokens], f32, True)

    # Partition-tiled views: row index r = ko*128 + p
    xT3 = xT[:].rearrange("(ko p) t -> p ko t", p=P)
    muT3 = muT[:].rearrange("(ko p) m -> p ko m", p=P)
    rhoT3 = rhoT[:].rearrange("(ko p) m -> p ko m", p=P)
    epsT3 = epsT[:].rearrange("(ko p) m -> p ko m", p=P)
    out3 = out[:].rearrange("(ms p) t -> p ms t", p=P)

    with tile.TileContext(nc) as tc:
        with (
            tc.tile_pool(name="wpool", bufs=1) as wpool,
            tc.tile_pool(name="bias", bufs=1) as bias_pool,
            tc.tile_pool(name="xpool", bufs=2) as xpool,
            tc.tile_pool(name="opool", bufs=6) as opool,
            tc.tile_pool(name="psum", bufs=8, space="PSUM") as psum_pool,
        ):
            # ---- bias: b = bias_mu + softplus(bias_rho) * eps_b  ([128, MS])
            bmu_t = bias_pool.tile([P, MS], f32, tag="bmu")
            nc.sync.dma_start(bmu_t[:], bmu[:])
            brho_t = bias_pool.tile([P, MS], f32, tag="brho")
            nc.sync.dma_start(brho_t[:], brho[:])
            beps_t = bias_pool.tile([P, MS], f32, tag="beps")
            nc.sync.dma_start(beps_t[:], beps[:])
            # softplus(v) = ln(exp(v) + 1) — Softplus has no ACT table on gen3
            b_sp = bias_pool.tile([P, MS], f32, tag="bsp")
            nc.scalar.activation(b_sp[:], brho_t[:], EXP)
            nc.scalar.activation(b_sp[:], b_sp[:], LN, bias=1.0)
            b_sb = bias_pool.tile([P, MS], f32, tag="bsb")
            nc.vector.tensor_mul(b_sb[:], b_sp[:], beps_t[:])
            nc.vector.tensor_add(b_sb[:], b_sb[:], bmu_t[:])

            # ---- weights: wT = mu + softplus(rho) * eps, downcast to bf16
            # wT layout [128, KO, o_sh]: lhsT tiles for the matmul.
            wT = wpool.tile([P, KO, o_sh], bf16, tag="wT")
            with tc.tile_pool(name="wtmp", bufs=2) as wtmp:
                for _wrep in range(wrepeat):
                    for kg in range(KO // KG):
                        sl = slice(kg * KG, (kg + 1) * KG)
                        rho_t = wtmp.tile([P, KG, o_sh], f32, tag="rho")
                        nc.sync.dma_start(rho_t[:], rhoT3[:, sl, :])
                        eps_t = wtmp.tile([P, KG, o_sh], f32, tag="eps")
                        nc.sync.dma_start(eps_t[:], epsT3[:, sl, :])
                        mu_t = wtmp.tile([P, KG, o_sh], f32, tag="mu")
                        nc.sync.dma_start(mu_t[:], muT3[:, sl, :])
                        sp_t = wtmp.tile([P, KG, o_sh], f32, tag="sp")
                        nc.scalar.activation(sp_t[:], rho_t[:], EXP)
                        nc.scalar.activation(sp_t[:], sp_t[:], LN, bias=1.0)
                        pr_t = wtmp.tile([P, KG, o_sh], f32, tag="pr")
                        nc.vector.tensor_mul(pr_t[:], sp_t[:], eps_t[:])
                        nc.vector.tensor_add(wT[:, sl, :], pr_t[:], mu_t[:])

            # ---- main loop: y^T[o, t] += w[o, i] * x[t, i]
            for _mrep in range(mrepeat):
                for n in range(NT):
                    tsl = slice(n * NF, (n + 1) * NF)
                    x_t = xpool.tile([P, KO, NF], bf16, tag="x")
                    nc.sync.dma_start(x_t[:], xT3[:, :, tsl])
                    for ms in range(MS):
                        ps = psum_pool.tile([P, NF], f32, tag="ps")
                        for ko in range(KO):
                            nc.tensor.matmul(
                                ps[:],
                                wT[:, ko : ko + 1, ms * P : (ms + 1) * P],
                                x_t[:, ko : ko + 1, :],
                                start=(ko == 0),
                                stop=(ko == KO - 1),
                            )
                        o_t = opool.tile([P, NF], f32, tag="o")
                        nc.scalar.activation(
                            o_t[:], ps[:], IDENT, bias=b_sb[:, ms : ms + 1], scale=1.0
                        )
                        nc.sync.dma_start(out3[:, ms, tsl], o_t[:])

    nc.compile()
    return nc


def shard_inputs(x, weight_mu, weight_rho, bias_mu, bias_rho, eps_w, eps_b,
                 in_f=IN_F, o_sh=O_SH, tokens=TOKENS, ncores=NCORES):
    """Host-side layout + sharding: transpose to [in, out] / [in, tokens]."""
    bf16 = ml_dtypes.bfloat16
    MS = o_sh // P
    xT_bf = np.ascontiguousarray(np.asarray(x, dtype=np.float32).astype(bf16).T)
    in_maps = []
    for c in range(ncores):
        sl = slice(c * o_sh, (c + 1) * o_sh)
        in_maps.append({
            "xT": xT_bf,
            "muT": np.ascontiguousarray(np.asarray(weight_mu)[sl, :].T),
            "rhoT": np.ascontiguousarray(np.asarray(weight_rho)[sl, :].T),
            "epsT": np.ascontiguousarray(np.asarray(eps_w)[sl, :].T),
            "bmu": np.ascontiguousarray(np.asarray(bias_mu)[sl].reshape(MS, P).T),
            "brho": np.ascontiguousarray(np.asarray(bias_rho)[sl].reshape(MS, P).T),
            "beps": np.ascontiguousarray(np.asarray(eps_b)[sl].reshape(MS, P).T),
        })
    return in_maps


_NC_CACHE = {}


def _get_nc():
    if "nc" not in _NC_CACHE:
        _NC_CACHE["nc"] = build_nc()
    return _NC_CACHE["nc"]


def kernel(x, weight_mu, weight_rho, bias_mu, bias_rho, eps_w, eps_b):
    from concourse import bass_utils

    nc = _get_nc()
    in_maps = shard_inputs(x, weight_mu, weight_rho, bias_mu, bias_rho, eps_w, eps_b)
    res = bass_utils.run_bass_kernel_spmd(nc, in_maps, core_ids=list(range(NCORES)))
    yT = np.concatenate([res.results[c]["out"] for c in range(NCORES)], axis=0)
    return np.ascontiguousarray(yT.T)


# revision 37
# speedup vs baseline: 140.8025x; 4.5840x over previous
"""Bayesian linear layer (reparameterized sampling) on 8 Trainium2 NeuronCores.

Computes y = x @ (mu + softplus(rho) * eps_w)^T + (bias_mu + softplus(bias_rho) * eps_b)
with x [8192, 4096], weights [4096, 4096].

Strategy: column-parallel tensor parallelism. Each of the 8 cores owns a
512-wide slice of out_features: it materializes its weight slice
w_c = mu_c + softplus(rho_c) * eps_c on-chip (ACT softplus + DVE mul/add
in bf16), then computes y_c^T = w_c @ x^T on the TensorEngine (bf16
matmul, fp32 PSUM accumulation), fusing the bias add into the
PSUM->SBUF copy on the vector engine. x is replicated to all cores as
bf16 in [in_features, tokens] layout so the contraction dim lands on
partitions with no on-chip transposes. Outputs stay sharded ([512, 8192]
per core) and are concatenated/transposed on the host.

The first two token-chunks accumulate k-outermost across 8 concurrently
open PSUM banks, so the PE consumes weight chunks as they stream in from
HBM instead of stalling until the whole weight slice is materialized.
"""

import sys

for _p in ("/opt/trn_rl_repo",):
    if _p not in sys.path:
        sys.path.insert(0, _p)

import numpy as np
import ml_dtypes

IN_F = 4096
OUT_F = 4096
TOKENS = 8192
NCORES = 8
O_SH = OUT_F // NCORES  # 512 out-features per core

P = 128
NF = 512  # matmul free dim (one PSUM bank of fp32)


def build_nc(in_f=IN_F, o_sh=O_SH, tokens=TOKENS, wrepeat=1, mrepeat=1):
    """Build the per-core Bass graph. All cores run the same graph (SPMD).

    wrepeat/mrepeat repeat the weight-precompute / main loop for
    slope-based benchmarking (identical redundant work); production is 1/1.
    """
    import concourse.bass as bass  # noqa: F401
    import concourse.mybir as mybir
    from concourse import bacc, tile

    f32 = mybir.dt.float32
    bf16 = mybir.dt.bfloat16
    KO = in_f // P        # k tiles of 128
    MS = o_sh // P        # psum-partition (out-feature) subtiles
    NT = tokens // NF     # token chunks
    KG = 2                # k tiles per weight-precompute chunk
    NSTREAM = min(2, NT)  # chunks computed k-outer while weights stream in
    EXP = mybir.ActivationFunctionType.Exp
    LN = mybir.ActivationFunctionType.Ln

    nc = bacc.Bacc(None, target_bir_lowering=False)

    NKG = in_f // P // KG  # packed weight chunks
    xT = nc.declare_dram_parameter("xT", [in_f, tokens], bf16, False)
    muT = nc.declare_dram_parameter("muT", [P, NKG, KG, o_sh], bf16, False)
    rhoT = nc.declare_dram_parameter("rhoT", [P, NKG, KG, o_sh], f32, False)
    epsT = nc.declare_dram_parameter("epsT", [P, NKG, KG, o_sh], bf16, False)
    bmu = nc.declare_dram_parameter("bmu", [P, MS], f32, False)
    brho = nc.declare_dram_parameter("brho", [P, MS], f32, False)
    beps = nc.declare_dram_parameter("beps", [P, MS], f32, False)
    out = nc.declare_dram_parameter("out", [o_sh, tokens], f32, True)

    # Partition-tiled views: row index r = ko*128 + p
    xT3 = xT[:].rearrange("(ko p) t -> p ko t", p=P)
    out3 = out[:].rearrange("(ms p) t -> p ms t", p=P)

    with tile.TileContext(nc) as tc:
        with (
            tc.tile_pool(name="wpool", bufs=1) as wpool,
            tc.tile_pool(name="bias", bufs=1) as bias_pool,
            tc.tile_pool(name="xpool", bufs=2) as xpool,
            tc.tile_pool(name="opool", bufs=4) as opool,
            tc.tile_pool(name="psum", bufs=8, space="PSUM") as psum_pool,
        ):
            # ---- bias: b = bias_mu + softplus(bias_rho) * eps_b  ([128, MS])
            # DMA ring split: x loads ride the Sync HWDGE ring, weight/bias
            # loads the Scalar HWDGE ring, out stores the GpSimd SWDGE path,
            # so the big weight stream never heads-of-line-blocks x chunks.
            bmu_t = bias_pool.tile([P, MS], f32, tag="bmu")
            nc.scalar.dma_start(bmu_t[:], bmu[:])
            brho_t = bias_pool.tile([P, MS], f32, tag="brho")
            nc.scalar.dma_start(brho_t[:], brho[:])
            beps_t = bias_pool.tile([P, MS], f32, tag="beps")
            nc.scalar.dma_start(beps_t[:], beps[:])
            # softplus(v) = ln(exp(v) + 1) — Softplus has no ACT table on
            # gen3; bias softplus is computed inside the weight phase so
            # its Exp/Ln share the batched table loads.
            b_sp = bias_pool.tile([P, MS], f32, tag="bsp")
            b_sb = bias_pool.tile([P, MS], f32, tag="bsb")

            # ---- x chunk loads: 4 piece-tiles per chunk so the first
            # matmuls start after ~1MB instead of the full 4MB load.
            NXP = 4
            KOP = KO // NXP

            def load_x(n):
                pieces = []
                for q in range(NXP):
                    xp = xpool.tile([P, KOP, NF], bf16, tag=f"x{q}",
                                    name=f"x_{n}_{q}")
                    nc.sync.dma_start(
                        xp[:], xT3[:, q * KOP : (q + 1) * KOP,
                                   n * NF : (n + 1) * NF])
                    pieces.append(xp)
                return pieces

            # ---- first streamed x chunks: issued before the weight DMAs
            xs = [load_x(n) for n in range(NSTREAM)]

            # ---- weights: wT = mu + softplus(rho) * eps (bf16)
            # Two passes over the chunks — all Exp ops, then all Ln ops —
            # so the ACT engine switches tables twice total instead of
            # paying a ~1.3us ACT_TABLE_LOAD on every alternation.
            # One wT tile per k-chunk: matmuls depend on individual
            # chunks, so the PE starts as soon as chunk 0 lands.
            wts = []
            QB = 4  # chunks per DMA / per exp-ln table batch
            with tc.tile_pool(name="spp", bufs=1) as spp, \
                 tc.tile_pool(name="wtmp", bufs=2) as wtmp:
                for _wrep in range(wrepeat):
                    bounds = [0, 1] + list(range(QB, NKG, QB)) + [NKG]
                    bounds = sorted(set(b for b in bounds if b <= NKG))
                    for qb, qe in zip(bounds[:-1], bounds[1:]):
                        nq = qe - qb
                        rho_q = wtmp.tile([P, nq, KG, o_sh], f32, tag="rho")
                        nc.scalar.dma_start(rho_q[:], rhoT[:][:, qb:qe, :, :])
                        eps_q = wtmp.tile([P, nq, KG, o_sh], bf16, tag="eps")
                        nc.scalar.dma_start(eps_q[:], epsT[:][:, qb:qe, :, :])
                        mu_q = wtmp.tile([P, nq, KG, o_sh], bf16, tag="mu")
                        nc.scalar.dma_start(mu_q[:], muT[:][:, qb:qe, :, :])
                        sps = {}
                        for kg in range(qb, qe):
                            sp_b = spp.tile([P, KG, o_sh], bf16,
                                            tag=f"spb{kg % QB}",
                                            name=f"spb_{kg}")
                            nc.scalar.activation(sp_b[:], rho_q[:, kg - qb],
                                                 EXP)
                            sps[kg] = sp_b
                        if qb == 0:
                            nc.scalar.activation(b_sp[:], brho_t[:], EXP)
                        for kg in range(qb, qe):
                            sp_l = wtmp.tile([P, KG, o_sh], bf16, tag="spl")
                            nc.scalar.activation(sp_l[:], sps[kg][:], LN,
                                                 bias=1.0)
                            pr_t = wtmp.tile([P, KG, o_sh], bf16, tag="pr")
                            nc.vector.tensor_mul(pr_t[:], sp_l[:],
                                                 eps_q[:, kg - qb])
                            w_t = wpool.tile([P, KG, o_sh], bf16,
                                             tag=f"wT{kg}")
                            nc.vector.tensor_add(w_t[:], pr_t[:],
                                                 mu_q[:, kg - qb])
                            if _wrep == 0:
                                wts.append(w_t)
                        if qb == 0:
                            nc.scalar.activation(b_sp[:], b_sp[:], LN,
                                                 bias=1.0)
                            nc.vector.tensor_mul(b_sb[:], b_sp[:], beps_t[:])
                            nc.vector.tensor_add(b_sb[:], b_sb[:], bmu_t[:])

            def close_group(ps, ms, n):
                o_t = opool.tile([P, NF], f32, tag="o")
                nc.vector.tensor_scalar_add(o_t[:], ps[:], b_sb[:, ms : ms + 1])
                nc.gpsimd.dma_start(
                    out3[:, ms, n * NF : (n + 1) * NF], o_t[:]
                )

            # ---- main loop: y^T[o, t] += w[o, i] * x[t, i]
            for _mrep in range(mrepeat):
                # Streaming prologue: NSTREAM chunks, k-outermost, so each
                # weight chunk is consumed on arrival (8 PSUM banks open).
                if _mrep > 0:
                    xs = [load_x(n) for n in range(NSTREAM)]
                pss = [[psum_pool.tile([P, NF], f32, tag="ps",
                                       name=f"ps_s{n}_{ms}")
                        for ms in range(MS)]
                       for n in range(NSTREAM)]
                for ko in range(KO):
                    w_sl = wts[ko // KG][:, ko % KG : ko % KG + 1, :]
                    for n in range(NSTREAM):
                        for ms in range(MS):
                            nc.tensor.matmul(
                                pss[n][ms][:],
                                w_sl[:, :, ms * P : (ms + 1) * P],
                                xs[n][ko // KOP][:, ko % KOP : ko % KOP + 1, :],
                                start=(ko == 0),
                                stop=(ko == KO - 1),
                            )
                for n in range(NSTREAM):
                    for ms in range(MS):
                        close_group(pss[n][ms], ms, n)

                # Steady state: weights resident; k-innermost (PE-dense).
                for n in range(NSTREAM, NT):
                    x_t = load_x(n)
                    for ms in range(MS):
                        ps = psum_pool.tile([P, NF], f32, tag="ps")
                        for ko in range(KO):
                            nc.tensor.matmul(
                                ps[:],
                                wts[ko // KG][:, ko % KG : ko % KG + 1,
                                              ms * P : (ms + 1) * P],
                                x_t[ko // KOP][:, ko % KOP : ko % KOP + 1, :],
                                start=(ko == 0),
                                stop=(ko == KO - 1),
                            )
                        close_group(ps, ms, n)

    nc.compile()
    return nc


def shard_inputs(x, weight_mu, weight_rho, bias_mu, bias_rho, eps_w, eps_b,
                 in_f=IN_F, o_sh=O_SH, tokens=TOKENS, ncores=NCORES):
    """Host-side layout + sharding: transpose to [in, out] / [in, tokens]."""
    bf16 = ml_dtypes.bfloat16
    MS = o_sh // P
    KG = 2
    KO = in_f // P
    xT_bf = np.ascontiguousarray(np.asarray(x, dtype=np.float32).astype(bf16).T)
    muT_bf = np.asarray(weight_mu, dtype=np.float32).astype(bf16)
    epsT_bf = np.asarray(eps_w, dtype=np.float32).astype(bf16)

    def pack_w(wt):
        # [in_f, o_sh] -> [P, KO//KG, KG, o_sh]; row r=(kg*KG+j)*128+p
        return np.ascontiguousarray(
            wt.reshape(KO // KG, KG, P, o_sh).transpose(2, 0, 1, 3))

    in_maps = []
    for c in range(ncores):
        sl = slice(c * o_sh, (c + 1) * o_sh)
        in_maps.append({
            "xT": xT_bf,
            "muT": pack_w(np.ascontiguousarray(muT_bf[sl, :].T)),
            "rhoT": pack_w(np.ascontiguousarray(np.asarray(weight_rho)[sl, :].T)),
            "epsT": pack_w(np.ascontiguousarray(epsT_bf[sl, :].T)),
            "bmu": np.ascontiguousarray(np.asarray(bias_mu)[sl].reshape(MS, P).T),
            "brho": np.ascontiguousarray(np.asarray(bias_rho)[sl].reshape(MS, P).T),
            "beps": np.ascontiguousarray(np.asarray(eps_b)[sl].reshape(MS, P).T),
        })
    return in_maps


_NC_CACHE = {}


def _get_nc():
    if "nc" not in _NC_CACHE:
        _NC_CACHE["nc"] = build_nc()
    return _NC_CACHE["nc"]


def kernel(x, weight_mu, weight_rho, bias_mu, bias_rho, eps_w, eps_b):
    from concourse import bass_utils

    nc = _get_nc()
    in_maps = shard_inputs(x, weight_mu, weight_rho, bias_mu, bias_rho, eps_w, eps_b)
    res = bass_utils.run_bass_kernel_spmd(nc, in_maps, core_ids=list(range(NCORES)))
    yT = np.concatenate([res.results[c]["out"] for c in range(NCORES)], axis=0)
    return np.ascontiguousarray(yT.T)


# revision 43
# speedup vs baseline: 144.6903x; 1.0276x over previous
"""Bayesian linear layer (reparameterized sampling) on 8 Trainium2 NeuronCores.

Computes y = x @ (mu + softplus(rho) * eps_w)^T + (bias_mu + softplus(bias_rho) * eps_b)
with x [8192, 4096], weights [4096, 4096].

Strategy: column-parallel tensor parallelism. Each of the 8 cores owns a
512-wide slice of out_features: it materializes its weight slice
w_c = mu_c + softplus(rho_c) * eps_c on-chip (ACT softplus + DVE mul/add
in bf16), then computes y_c^T = w_c @ x^T on the TensorEngine (bf16
matmul, fp32 PSUM accumulation), fusing the bias add into the
PSUM->SBUF copy on the vector engine. x is replicated to all cores as
bf16 in [in_features, tokens] layout so the contraction dim lands on
partitions with no on-chip transposes. Outputs stay sharded ([512, 8192]
per core) and are concatenated/transposed on the host.

The first two token-chunks accumulate k-outermost across 8 concurrently
open PSUM banks, so the PE consumes weight chunks as they stream in from
HBM instead of stalling until the whole weight slice is materialized.
"""

import sys

for _p in ("/opt/trn_rl_repo",):
    if _p not in sys.path:
        sys.path.insert(0, _p)

import numpy as np
import ml_dtypes

IN_F = 4096
OUT_F = 4096
TOKENS = 8192
NCORES = 8
O_SH = OUT_F // NCORES  # 512 out-features per core

P = 128
NF = 512  # matmul free dim (one PSUM bank of fp32)


def build_nc(in_f=IN_F, o_sh=O_SH, tokens=TOKENS, wrepeat=1, mrepeat=1):
    """Build the per-core Bass graph. All cores run the same graph (SPMD).

    wrepeat/mrepeat repeat the weight-precompute / main loop for
    slope-based benchmarking (identical redundant work); production is 1/1.
    """
    import concourse.bass as bass  # noqa: F401
    import concourse.mybir as mybir
    from concourse import bacc, tile

    f32 = mybir.dt.float32
    bf16 = mybir.dt.bfloat16
    f16 = mybir.dt.float16
    KO = in_f // P        # k tiles of 128
    MS = o_sh // P        # psum-partition (out-feature) subtiles
    NT = tokens // NF     # token chunks
    KG = 2                # k tiles per weight-precompute chunk
    NSTREAM = min(2, NT)  # chunks computed k-outer while weights stream in
    EXP = mybir.ActivationFunctionType.Exp
    LN = mybir.ActivationFunctionType.Ln

    nc = bacc.Bacc(None, target_bir_lowering=False)

    NKG = in_f // P // KG  # packed weight chunks
    xT = nc.declare_dram_parameter("xT", [in_f, tokens], bf16, False)
    muT = nc.declare_dram_parameter("muT", [P, NKG, KG, o_sh], bf16, False)
    rhoT = nc.declare_dram_parameter("rhoT", [P, NKG, KG, o_sh], f16, False)
    epsT = nc.declare_dram_parameter("epsT", [P, NKG, KG, o_sh], bf16, False)
    bmu = nc.declare_dram_parameter("bmu", [P, MS], f32, False)
    brho = nc.declare_dram_parameter("brho", [P, MS], f32, False)
    beps = nc.declare_dram_parameter("beps", [P, MS], f32, False)
    out = nc.declare_dram_parameter("out", [o_sh, tokens], f32, True)

    # Partition-tiled views: row index r = ko*128 + p
    xT3 = xT[:].rearrange("(ko p) t -> p ko t", p=P)
    out3 = out[:].rearrange("(ms p) t -> p ms t", p=P)

    with tile.TileContext(nc) as tc:
        with (
            tc.tile_pool(name="wpool", bufs=1) as wpool,
            tc.tile_pool(name="bias", bufs=1) as bias_pool,
            tc.tile_pool(name="xpool", bufs=2) as xpool,
            tc.tile_pool(name="opool", bufs=4) as opool,
            tc.tile_pool(name="psum", bufs=8, space="PSUM") as psum_pool,
        ):
            # ---- bias: b = bias_mu + softplus(bias_rho) * eps_b  ([128, MS])
            # DMA ring split: x loads ride the Sync HWDGE ring, weight/bias
            # loads the Scalar HWDGE ring, out stores the GpSimd SWDGE path,
            # so the big weight stream never heads-of-line-blocks x chunks.
            bmu_t = bias_pool.tile([P, MS], f32, tag="bmu")
            nc.scalar.dma_start(bmu_t[:], bmu[:])
            brho_t = bias_pool.tile([P, MS], f32, tag="brho")
            nc.scalar.dma_start(brho_t[:], brho[:])
            beps_t = bias_pool.tile([P, MS], f32, tag="beps")
            nc.scalar.dma_start(beps_t[:], beps[:])
            # softplus(v) = ln(exp(v) + 1) — Softplus has no ACT table on
            # gen3; bias softplus is computed inside the weight phase so
            # its Exp/Ln share the batched table loads.
            b_sp = bias_pool.tile([P, MS], f32, tag="bsp")
            b_sb = bias_pool.tile([P, MS], f32, tag="bsb")

            # ---- x chunk loads: 4 piece-tiles per chunk so the first
            # matmuls start after ~1MB instead of the full 4MB load.
            NXP = 4
            KOP = KO // NXP

            def load_x(n):
                pieces = []
                for q in range(NXP):
                    xp = xpool.tile([P, KOP, NF], bf16, tag=f"x{q}",
                                    name=f"x_{n}_{q}")
                    nc.sync.dma_start(
                        xp[:], xT3[:, q * KOP : (q + 1) * KOP,
                                   n * NF : (n + 1) * NF])
                    pieces.append(xp)
                return pieces

            # ---- first streamed x chunks: issued before the weight DMAs
            xs = [load_x(n) for n in range(NSTREAM)]

            # ---- weights: wT = mu + softplus(rho) * eps (bf16)
            # Two passes over the chunks — all Exp ops, then all Ln ops —
            # so the ACT engine switches tables twice total instead of
            # paying a ~1.3us ACT_TABLE_LOAD on every alternation.
            # One wT tile per k-chunk: matmuls depend on individual
            # chunks, so the PE starts as soon as chunk 0 lands.
            wts = []
            QB = 4  # chunks per DMA / per exp-ln table batch
            with tc.tile_pool(name="spp", bufs=1) as spp, \
                 tc.tile_pool(name="wtmp", bufs=2) as wtmp:
                for _wrep in range(wrepeat):
                    # table batches (exp x all, then ln x all) over dma groups
                    batches = [[(0, 1)],
                               [(1, 4), (4, 8)],
                               [(8, 12), (12, NKG)]]
                    if NKG <= 4:  # small problem sizes (sim)
                        batches = [[(0, 1)], [(1, NKG)]] if NKG > 1 else [[(0, 1)]]
                    first = True
                    for batch in batches:
                        rqs, eqs, mqs = {}, {}, {}
                        for qb, qe in batch:
                            nq = qe - qb
                            rho_q = wtmp.tile([P, nq, KG, o_sh], f16, tag="rho",
                                              name=f"rho_{qb}")
                            nc.scalar.dma_start(rho_q[:], rhoT[:][:, qb:qe, :, :])
                            eps_q = wtmp.tile([P, nq, KG, o_sh], bf16, tag="eps",
                                              name=f"eps_{qb}")
                            nc.scalar.dma_start(eps_q[:], epsT[:][:, qb:qe, :, :])
                            mu_q = wtmp.tile([P, nq, KG, o_sh], bf16, tag="mu",
                                             name=f"mu_{qb}")
                            nc.scalar.dma_start(mu_q[:], muT[:][:, qb:qe, :, :])
                            rqs[qb], eqs[qb], mqs[qb] = rho_q, eps_q, mu_q
                        sps = {}
                        for qb, qe in batch:
                            for kg in range(qb, qe):
                                sp_b = spp.tile([P, KG, o_sh], bf16,
                                                tag=f"spb{kg % 8}",
                                                name=f"spb_{kg}")
                                nc.scalar.activation(sp_b[:],
                                                     rqs[qb][:, kg - qb], EXP)
                                sps[kg] = sp_b
                        if first:
                            nc.scalar.activation(b_sp[:], brho_t[:], EXP)
                        for qb, qe in batch:
                            for kg in range(qb, qe):
                                sp_l = wtmp.tile([P, KG, o_sh], bf16, tag="spl")
                                nc.scalar.activation(sp_l[:], sps[kg][:], LN,
                                                     bias=1.0)
                                pr_t = wtmp.tile([P, KG, o_sh], bf16, tag="pr")
                                nc.vector.tensor_mul(pr_t[:], sp_l[:],
                                                     eqs[qb][:, kg - qb])
                                w_t = wpool.tile([P, KG, o_sh], bf16,
                                                 tag=f"wT{kg}")
                                nc.vector.tensor_add(w_t[:], pr_t[:],
                                                     mqs[qb][:, kg - qb])
                                if _wrep == 0:
                                    wts.append(w_t)
                        if first:
                            nc.scalar.activation(b_sp[:], b_sp[:], LN,
                                                 bias=1.0)
                            nc.vector.tensor_mul(b_sb[:], b_sp[:], beps_t[:])
                            nc.vector.tensor_add(b_sb[:], b_sb[:], bmu_t[:])
                            first = False

            def close_group(ps, ms, n):
                o_t = opool.tile([P, NF], f32, tag="o")
                nc.vector.tensor_scalar_add(o_t[:], ps[:], b_sb[:, ms : ms + 1])
                nc.gpsimd.dma_start(
                    out3[:, ms, n * NF : (n + 1) * NF], o_t[:]
                )

            # ---- main loop: y^T[o, t] += w[o, i] * x[t, i]
            for _mrep in range(mrepeat):
                # Streaming prologue: NSTREAM chunks, k-outermost, so each
                # weight chunk is consumed on arrival (8 PSUM banks open).
                if _mrep > 0:
                    xs = [load_x(n) for n in range(NSTREAM)]
                pss = [[psum_pool.tile([P, NF], f32, tag="ps",
                                       name=f"ps_s{n}_{ms}")
                        for ms in range(MS)]
                       for n in range(NSTREAM)]
                for ko in range(KO):
                    w_sl = wts[ko // KG][:, ko % KG : ko % KG + 1, :]
                    for n in range(NSTREAM):
                        for ms in range(MS):
                            nc.tensor.matmul(
                                pss[n][ms][:],
                                w_sl[:, :, ms * P : (ms + 1) * P],
                                xs[n][ko // KOP][:, ko % KOP : ko % KOP + 1, :],
                                start=(ko == 0),
                                stop=(ko == KO - 1),
                            )
                for n in range(NSTREAM):
                    for ms in range(MS):
                        close_group(pss[n][ms], ms, n)

                # Steady state: weights resident; k-innermost (PE-dense).
                for n in range(NSTREAM, NT):
                    x_t = load_x(n)
                    for ms in range(MS):
                        ps = psum_pool.tile([P, NF], f32, tag="ps")
                        for ko in range(KO):
                            nc.tensor.matmul(
                                ps[:],
                                wts[ko // KG][:, ko % KG : ko % KG + 1,
                                              ms * P : (ms + 1) * P],
                                x_t[ko // KOP][:, ko % KOP : ko % KOP + 1, :],
                                start=(ko == 0),
                                stop=(ko == KO - 1),
                            )
                        close_group(ps, ms, n)

    nc.compile()
    return nc


def shard_inputs(x, weight_mu, weight_rho, bias_mu, bias_rho, eps_w, eps_b,
                 in_f=IN_F, o_sh=O_SH, tokens=TOKENS, ncores=NCORES):
    """Host-side layout + sharding: transpose to [in, out] / [in, tokens]."""
    bf16 = ml_dtypes.bfloat16
    MS = o_sh // P
    KG = 2
    KO = in_f // P
    xT_bf = np.ascontiguousarray(np.asarray(x, dtype=np.float32).astype(bf16).T)
    muT_bf = np.asarray(weight_mu, dtype=np.float32).astype(bf16)
    epsT_bf = np.asarray(eps_w, dtype=np.float32).astype(bf16)

    def pack_w(wt):
        # [in_f, o_sh] -> [P, KO//KG, KG, o_sh]; row r=(kg*KG+j)*128+p
        return np.ascontiguousarray(
            wt.reshape(KO // KG, KG, P, o_sh).transpose(2, 0, 1, 3))

    in_maps = []
    for c in range(ncores):
        sl = slice(c * o_sh, (c + 1) * o_sh)
        in_maps.append({
            "xT": xT_bf,
            "muT": pack_w(np.ascontiguousarray(muT_bf[sl, :].T)),
            "rhoT": pack_w(np.ascontiguousarray(
                np.asarray(weight_rho)[sl, :].T.astype(np.float16))),
            "epsT": pack_w(np.ascontiguousarray(epsT_bf[sl, :].T)),
            "bmu": np.ascontiguousarray(np.asarray(bias_mu)[sl].reshape(MS, P).T),
            "brho": np.ascontiguousarray(np.asarray(bias_rho)[sl].reshape(MS, P).T),
            "beps": np.ascontiguousarray(np.asarray(eps_b)[sl].reshape(MS, P).T),
        })
    return in_maps


_NC_CACHE = {}


def _get_nc():
    if "nc" not in _NC_CACHE:
        _NC_CACHE["nc"] = build_nc()
    return _NC_CACHE["nc"]


def kernel(x, weight_mu, weight_rho, bias_mu, bias_rho, eps_w, eps_b):
    from concourse import bass_utils

    nc = _get_nc()
    in_maps = shard_inputs(x, weight_mu, weight_rho, bias_mu, bias_rho, eps_w, eps_b)
    res = bass_utils.run_bass_kernel_spmd(nc, in_maps, core_ids=list(range(NCORES)))
    yT = np.concatenate([res.results[c]["out"] for c in range(NCORES)], axis=0)
    return np.ascontiguousarray(yT.T)


# revision 44
# speedup vs baseline: 146.4097x; 1.0119x over previous
"""Bayesian linear layer (reparameterized sampling) on 8 Trainium2 NeuronCores.

Computes y = x @ (mu + softplus(rho) * eps_w)^T + (bias_mu + softplus(bias_rho) * eps_b)
with x [8192, 4096], weights [4096, 4096].

Strategy: column-parallel tensor parallelism. Each of the 8 cores owns a
512-wide slice of out_features: it materializes its weight slice
w_c = mu_c + softplus(rho_c) * eps_c on-chip (ACT softplus + DVE mul/add
in bf16), then computes y_c^T = w_c @ x^T on the TensorEngine (bf16
matmul, fp32 PSUM accumulation), fusing the bias add into the
PSUM->SBUF copy on the vector engine. x is replicated to all cores as
bf16 in [in_features, tokens] layout so the contraction dim lands on
partitions with no on-chip transposes. Outputs stay sharded ([512, 8192]
per core) and are concatenated/transposed on the host.

The first two token-chunks accumulate k-outermost across 8 concurrently
open PSUM banks, so the PE consumes weight chunks as they stream in from
HBM instead of stalling until the whole weight slice is materialized.
"""

import sys

for _p in ("/opt/trn_rl_repo",):
    if _p not in sys.path:
        sys.path.insert(0, _p)

import numpy as np
import ml_dtypes

IN_F = 4096
OUT_F = 4096
TOKENS = 8192
NCORES = 8
O_SH = OUT_F // NCORES  # 512 out-features per core

P = 128
NF = 512  # matmul free dim (one PSUM bank of fp32)


def build_nc(in_f=IN_F, o_sh=O_SH, tokens=TOKENS, wrepeat=1, mrepeat=1):
    """Build the per-core Bass graph. All cores run the same graph (SPMD).

    wrepeat/mrepeat repeat the weight-precompute / main loop for
    slope-based benchmarking (identical redundant work); production is 1/1.
    """
    import concourse.bass as bass  # noqa: F401
    import concourse.mybir as mybir
    from concourse import bacc, tile

    f32 = mybir.dt.float32
    bf16 = mybir.dt.bfloat16
    f16 = mybir.dt.float16
    KO = in_f // P        # k tiles of 128
    MS = o_sh // P        # psum-partition (out-feature) subtiles
    NT = tokens // NF     # token chunks
    KG = 2                # k tiles per weight-precompute chunk
    NSTREAM = min(2, NT)  # chunks computed k-outer while weights stream in
    EXP = mybir.ActivationFunctionType.Exp
    LN = mybir.ActivationFunctionType.Ln

    nc = bacc.Bacc(None, target_bir_lowering=False)

    NKG = in_f // P // KG  # packed weight chunks
    xT = nc.declare_dram_parameter("xT", [in_f, tokens], bf16, False)
    muT = nc.declare_dram_parameter("muT", [P, NKG, KG, o_sh], bf16, False)
    rhoT = nc.declare_dram_parameter("rhoT", [P, NKG, KG, o_sh], f16, False)
    epsT = nc.declare_dram_parameter("epsT", [P, NKG, KG, o_sh], bf16, False)
    bmu = nc.declare_dram_parameter("bmu", [P, MS], f32, False)
    brho = nc.declare_dram_parameter("brho", [P, MS], f32, False)
    beps = nc.declare_dram_parameter("beps", [P, MS], f32, False)
    out = nc.declare_dram_parameter("out", [o_sh, tokens], f32, True)

    # Partition-tiled views: row index r = ko*128 + p
    xT3 = xT[:].rearrange("(ko p) t -> p ko t", p=P)
    out3 = out[:].rearrange("(ms p) t -> p ms t", p=P)

    with tile.TileContext(nc) as tc:
        with (
            tc.tile_pool(name="wpool", bufs=1) as wpool,
            tc.tile_pool(name="bias", bufs=1) as bias_pool,
            tc.tile_pool(name="xpool", bufs=2) as xpool,
            tc.tile_pool(name="opool", bufs=4) as opool,
            tc.tile_pool(name="psum", bufs=8, space="PSUM") as psum_pool,
        ):
            # ---- bias: b = bias_mu + softplus(bias_rho) * eps_b  ([128, MS])
            # DMA ring split: x loads ride the Sync HWDGE ring, weight/bias
            # loads the Scalar HWDGE ring, out stores the GpSimd SWDGE path,
            # so the big weight stream never heads-of-line-blocks x chunks.
            bmu_t = bias_pool.tile([P, MS], f32, tag="bmu")
            nc.scalar.dma_start(bmu_t[:], bmu[:])
            brho_t = bias_pool.tile([P, MS], f32, tag="brho")
            nc.scalar.dma_start(brho_t[:], brho[:])
            beps_t = bias_pool.tile([P, MS], f32, tag="beps")
            nc.scalar.dma_start(beps_t[:], beps[:])
            # softplus(v) = ln(exp(v) + 1) — Softplus has no ACT table on
            # gen3; bias softplus is computed inside the weight phase so
            # its Exp/Ln share the batched table loads.
            b_sp = bias_pool.tile([P, MS], f32, tag="bsp")
            b_sb = bias_pool.tile([P, MS], f32, tag="bsb")

            # ---- x chunk loads: 4 piece-tiles per chunk so the first
            # matmuls start after ~1MB instead of the full 4MB load.
            NXP = 4
            KOP = KO // NXP

            def load_x(n):
                pieces = []
                for q in range(NXP):
                    xp = xpool.tile([P, KOP, NF], bf16, tag=f"x{q}",
                                    name=f"x_{n}_{q}")
                    nc.sync.dma_start(
                        xp[:], xT3[:, q * KOP : (q + 1) * KOP,
                                   n * NF : (n + 1) * NF])
                    pieces.append(xp)
                return pieces

            # ---- first streamed x chunks: issued before the weight DMAs
            xs = [load_x(n) for n in range(NSTREAM)]

            # ---- weights: wT = mu + softplus(rho) * eps (bf16)
            # Two passes over the chunks — all Exp ops, then all Ln ops —
            # so the ACT engine switches tables twice total instead of
            # paying a ~1.3us ACT_TABLE_LOAD on every alternation.
            # One wT tile per k-chunk: matmuls depend on individual
            # chunks, so the PE starts as soon as chunk 0 lands.
            wts = []
            QB = 4  # chunks per DMA / per exp-ln table batch
            with tc.tile_pool(name="spp", bufs=1) as spp, \
                 tc.tile_pool(name="wtmp", bufs=2) as wtmp:
                for _wrep in range(wrepeat):
                    # table batches (exp x all, then ln x all) over dma groups
                    batches = [[(0, 1)],
                               [(1, 4), (4, 8)],
                               [(8, 12), (12, NKG)]]
                    if NKG <= 4:  # small problem sizes (sim)
                        batches = [[(0, 1)], [(1, NKG)]] if NKG > 1 else [[(0, 1)]]
                    first = True
                    for batch in batches:
                        rqs, eqs, mqs = {}, {}, {}
                        for qb, qe in batch:
                            nq = qe - qb
                            rho_q = wtmp.tile([P, nq, KG, o_sh], f16, tag="rho",
                                              name=f"rho_{qb}")
                            nc.scalar.dma_start(rho_q[:], rhoT[:][:, qb:qe, :, :])
                            eps_q = wtmp.tile([P, nq, KG, o_sh], bf16, tag="eps",
                                              name=f"eps_{qb}")
                            nc.scalar.dma_start(eps_q[:], epsT[:][:, qb:qe, :, :])
                            mu_q = wtmp.tile([P, nq, KG, o_sh], bf16, tag="mu",
                                             name=f"mu_{qb}")
                            nc.scalar.dma_start(mu_q[:], muT[:][:, qb:qe, :, :])
                            rqs[qb], eqs[qb], mqs[qb] = rho_q, eps_q, mu_q
                        sps = {}
                        for qb, qe in batch:
                            for kg in range(qb, qe):
                                sp_b = spp.tile([P, KG, o_sh], bf16,
                                                tag=f"spb{kg % 8}",
                                                name=f"spb_{kg}")
                                nc.scalar.activation(sp_b[:],
                                                     rqs[qb][:, kg - qb], EXP)
                                sps[kg] = sp_b
                        if first:
                            nc.scalar.activation(b_sp[:], brho_t[:], EXP)
                        for qb, qe in batch:
                            for kg in range(qb, qe):
                                sp_l = wtmp.tile([P, KG, o_sh], bf16, tag="spl")
                                nc.scalar.activation(sp_l[:], sps[kg][:], LN,
                                                     bias=1.0)
                                pr_t = wtmp.tile([P, KG, o_sh], bf16, tag="pr")
                                nc.vector.tensor_mul(pr_t[:], sp_l[:],
                                                     eqs[qb][:, kg - qb])
                                w_t = wpool.tile([P, KG, o_sh], bf16,
                                                 tag=f"wT{kg}")
                                nc.vector.tensor_add(w_t[:], pr_t[:],
                                                     mqs[qb][:, kg - qb])
                                if _wrep == 0:
                                    wts.append(w_t)
                        if first:
                            nc.scalar.activation(b_sp[:], b_sp[:], LN,
                                                 bias=1.0)
                            nc.vector.tensor_mul(b_sb[:], b_sp[:], beps_t[:])
                            nc.vector.tensor_add(b_sb[:], b_sb[:], bmu_t[:])
                            first = False

            def close_group(ps, ms, n):
                o_t = opool.tile([P, NF], f32, tag="o")
                nc.vector.tensor_scalar_add(o_t[:], ps[:], b_sb[:, ms : ms + 1])
                # scalar HWDGE ring is idle once the weight stream ends
                nc.scalar.dma_start(
                    out3[:, ms, n * NF : (n + 1) * NF], o_t[:]
                )

            # ---- main loop: y^T[o, t] += w[o, i] * x[t, i]
            for _mrep in range(mrepeat):
                # Streaming prologue: NSTREAM chunks, k-outermost, so each
                # weight chunk is consumed on arrival (8 PSUM banks open).
                if _mrep > 0:
                    xs = [load_x(n) for n in range(NSTREAM)]
                pss = [[psum_pool.tile([P, NF], f32, tag="ps",
                                       name=f"ps_s{n}_{ms}")
                        for ms in range(MS)]
                       for n in range(NSTREAM)]
                for ko in range(KO):
                    w_sl = wts[ko // KG][:, ko % KG : ko % KG + 1, :]
                    for n in range(NSTREAM):
                        for ms in range(MS):
                            nc.tensor.matmul(
                                pss[n][ms][:],
                                w_sl[:, :, ms * P : (ms + 1) * P],
                                xs[n][ko // KOP][:, ko % KOP : ko % KOP + 1, :],
                                start=(ko == 0),
                                stop=(ko == KO - 1),
                            )
                for n in range(NSTREAM):
                    for ms in range(MS):
                        close_group(pss[n][ms], ms, n)

                # Steady state: weights resident; k-innermost (PE-dense).
                for n in range(NSTREAM, NT):
                    x_t = load_x(n)
                    for ms in range(MS):
                        ps = psum_pool.tile([P, NF], f32, tag="ps")
                        for ko in range(KO):
                            nc.tensor.matmul(
                                ps[:],
                                wts[ko // KG][:, ko % KG : ko % KG + 1,
                                              ms * P : (ms + 1) * P],
                                x_t[ko // KOP][:, ko % KOP : ko % KOP + 1, :],
                                start=(ko == 0),
                                stop=(ko == KO - 1),
                            )
                        close_group(ps, ms, n)

    nc.compile()
    return nc


def shard_inputs(x, weight_mu, weight_rho, bias_mu, bias_rho, eps_w, eps_b,
                 in_f=IN_F, o_sh=O_SH, tokens=TOKENS, ncores=NCORES):
    """Host-side layout + sharding: transpose to [in, out] / [in, tokens]."""
    bf16 = ml_dtypes.bfloat16
    MS = o_sh // P
    KG = 2
    KO = in_f // P
    xT_bf = np.ascontiguousarray(np.asarray(x, dtype=np.float32).astype(bf16).T)
    muT_bf = np.asarray(weight_mu, dtype=np.float32).astype(bf16)
    epsT_bf = np.asarray(eps_w, dtype=np.float32).astype(bf16)

    def pack_w(wt):
        # [in_f, o_sh] -> [P, KO//KG, KG, o_sh]; row r=(kg*KG+j)*128+p
        return np.ascontiguousarray(
            wt.reshape(KO // KG, KG, P, o_sh).transpose(2, 0, 1, 3))

    in_maps = []
    for c in range(ncores):
        sl = slice(c * o_sh, (c + 1) * o_sh)
        in_maps.append({
            "xT": xT_bf,
            "muT": pack_w(np.ascontiguousarray(muT_bf[sl, :].T)),
            "rhoT": pack_w(np.ascontiguousarray(
                np.asarray(weight_rho)[sl, :].T.astype(np.float16))),
            "epsT": pack_w(np.ascontiguousarray(epsT_bf[sl, :].T)),
            "bmu": np.ascontiguousarray(np.asarray(bias_mu)[sl].reshape(MS, P).T),
            "brho": np.ascontiguousarray(np.asarray(bias_rho)[sl].reshape(MS, P).T),
            "beps": np.ascontiguousarray(np.asarray(eps_b)[sl].reshape(MS, P).T),
        })
    return in_maps


_NC_CACHE = {}


def _get_nc():
    if "nc" not in _NC_CACHE:
        _NC_CACHE["nc"] = build_nc()
    return _NC_CACHE["nc"]


def kernel(x, weight_mu, weight_rho, bias_mu, bias_rho, eps_w, eps_b):
    from concourse import bass_utils

    nc = _get_nc()
    in_maps = shard_inputs(x, weight_mu, weight_rho, bias_mu, bias_rho, eps_w, eps_b)
    res = bass_utils.run_bass_kernel_spmd(nc, in_maps, core_ids=list(range(NCORES)))
    yT = np.concatenate([res.results[c]["out"] for c in range(NCORES)], axis=0)
    return np.ascontiguousarray(yT.T)


# revision 46
# speedup vs baseline: 146.7246x; 1.0022x over previous
"""Bayesian linear layer (reparameterized sampling) on 8 Trainium2 NeuronCores.

Computes y = x @ (mu + softplus(rho) * eps_w)^T + (bias_mu + softplus(bias_rho) * eps_b)
with x [8192, 4096], weights [4096, 4096].

Strategy: column-parallel tensor parallelism. Each of the 8 cores owns a
512-wide slice of out_features: it materializes its weight slice
w_c = mu_c + softplus(rho_c) * eps_c on-chip (ACT softplus + DVE mul/add
in bf16), then computes y_c^T = w_c @ x^T on the TensorEngine (bf16
matmul, fp32 PSUM accumulation), fusing the bias add into the
PSUM->SBUF copy on the vector engine. x is replicated to all cores as
bf16 in [in_features, tokens] layout so the contraction dim lands on
partitions with no on-chip transposes. Outputs stay sharded ([512, 8192]
per core) and are concatenated/transposed on the host.

The first two token-chunks accumulate k-outermost across 8 concurrently
open PSUM banks, so the PE consumes weight chunks as they stream in from
HBM instead of stalling until the whole weight slice is materialized.
"""

import sys

for _p in ("/opt/trn_rl_repo",):
    if _p not in sys.path:
        sys.path.insert(0, _p)

import numpy as np
import ml_dtypes

IN_F = 4096
OUT_F = 4096
TOKENS = 8192
NCORES = 8
O_SH = OUT_F // NCORES  # 512 out-features per core

P = 128
NF = 512  # matmul free dim (one PSUM bank of fp32)


def build_nc(in_f=IN_F, o_sh=O_SH, tokens=TOKENS, wrepeat=1, mrepeat=1):
    """Build the per-core Bass graph. All cores run the same graph (SPMD).

    wrepeat/mrepeat repeat the weight-precompute / main loop for
    slope-based benchmarking (identical redundant work); production is 1/1.
    """
    import concourse.bass as bass  # noqa: F401
    import concourse.mybir as mybir
    from concourse import bacc, tile

    f32 = mybir.dt.float32
    bf16 = mybir.dt.bfloat16
    f16 = mybir.dt.float16
    KO = in_f // P        # k tiles of 128
    MS = o_sh // P        # psum-partition (out-feature) subtiles
    NT = tokens // NF     # token chunks
    KG = 2                # k tiles per weight-precompute chunk
    NSTREAM = min(2, NT)  # chunks computed k-outer while weights stream in
    EXP = mybir.ActivationFunctionType.Exp
    LN = mybir.ActivationFunctionType.Ln

    nc = bacc.Bacc(None, target_bir_lowering=False)

    NKG = in_f // P // KG  # packed weight chunks
    xT = nc.declare_dram_parameter("xT", [in_f, tokens], bf16, False)
    muT = nc.declare_dram_parameter("muT", [P, NKG, KG, o_sh], bf16, False)
    rhoT = nc.declare_dram_parameter("rhoT", [P, NKG, KG, o_sh], f16, False)
    epsT = nc.declare_dram_parameter("epsT", [P, NKG, KG, o_sh], bf16, False)
    bmu = nc.declare_dram_parameter("bmu", [P, MS], f32, False)
    brho = nc.declare_dram_parameter("brho", [P, MS], f32, False)
    beps = nc.declare_dram_parameter("beps", [P, MS], f32, False)
    out = nc.declare_dram_parameter("out", [o_sh, tokens], f32, True)

    # Partition-tiled views: row index r = ko*128 + p
    xT3 = xT[:].rearrange("(ko p) t -> p ko t", p=P)
    out3 = out[:].rearrange("(ms p) t -> p ms t", p=P)

    with tile.TileContext(nc) as tc:
        with (
            tc.tile_pool(name="wpool", bufs=1) as wpool,
            tc.tile_pool(name="bias", bufs=1) as bias_pool,
            tc.tile_pool(name="xpool", bufs=2) as xpool,
            tc.tile_pool(name="opool", bufs=4) as opool,
            tc.tile_pool(name="psum", bufs=8, space="PSUM") as psum_pool,
        ):
            # ---- bias: b = bias_mu + softplus(bias_rho) * eps_b  ([128, MS])
            # DMA ring split: x loads ride the Sync HWDGE ring, weight/bias
            # loads the Scalar HWDGE ring, out stores the GpSimd SWDGE path,
            # so the big weight stream never heads-of-line-blocks x chunks.
            bmu_t = bias_pool.tile([P, MS], f32, tag="bmu")
            nc.scalar.dma_start(bmu_t[:], bmu[:])
            brho_t = bias_pool.tile([P, MS], f32, tag="brho")
            nc.scalar.dma_start(brho_t[:], brho[:])
            beps_t = bias_pool.tile([P, MS], f32, tag="beps")
            nc.scalar.dma_start(beps_t[:], beps[:])
            # softplus(v) = ln(exp(v) + 1) — Softplus has no ACT table on
            # gen3; bias softplus is computed inside the weight phase so
            # its Exp/Ln share the batched table loads.
            b_sp = bias_pool.tile([P, MS], f32, tag="bsp")
            b_sb = bias_pool.tile([P, MS], f32, tag="bsb")

            # ---- x chunk loads: 4 piece-tiles per chunk so the first
            # matmuls start after ~1MB instead of the full 4MB load.
            NXP = 4
            KOP = KO // NXP

            def load_x(n):
                pieces = []
                for q in range(NXP):
                    xp = xpool.tile([P, KOP, NF], bf16, tag=f"x{q}",
                                    name=f"x_{n}_{q}")
                    nc.sync.dma_start(
                        xp[:], xT3[:, q * KOP : (q + 1) * KOP,
                                   n * NF : (n + 1) * NF])
                    pieces.append(xp)
                return pieces

            # ---- first streamed x chunks: issued before the weight DMAs
            xs = [load_x(n) for n in range(NSTREAM)]

            # ---- weights: wT = mu + softplus(rho) * eps (bf16)
            # Two passes over the chunks — all Exp ops, then all Ln ops —
            # so the ACT engine switches tables twice total instead of
            # paying a ~1.3us ACT_TABLE_LOAD on every alternation.
            # One wT tile per k-chunk: matmuls depend on individual
            # chunks, so the PE starts as soon as chunk 0 lands.
            wts = []
            QB = 4  # chunks per DMA / per exp-ln table batch
            with tc.tile_pool(name="spp", bufs=1) as spp, \
                 tc.tile_pool(name="wtmp", bufs=2) as wtmp:
                for _wrep in range(wrepeat):
                    # table batches (exp x all, then ln x all) over dma groups
                    batches = [[(0, 1)],
                               [(1, 4), (4, 8)],
                               [(8, 12), (12, NKG)]]
                    if NKG <= 4:  # small problem sizes (sim)
                        batches = [[(0, 1)], [(1, NKG)]] if NKG > 1 else [[(0, 1)]]
                    first = True
                    for batch in batches:
                        rqs, eqs, mqs = {}, {}, {}
                        for qb, qe in batch:
                            nq = qe - qb
                            rho_q = wtmp.tile([P, nq, KG, o_sh], f16, tag="rho",
                                              name=f"rho_{qb}")
                            nc.scalar.dma_start(rho_q[:], rhoT[:][:, qb:qe, :, :])
                            eps_q = wtmp.tile([P, nq, KG, o_sh], bf16, tag="eps",
                                              name=f"eps_{qb}")
                            nc.scalar.dma_start(eps_q[:], epsT[:][:, qb:qe, :, :])
                            mu_q = wtmp.tile([P, nq, KG, o_sh], bf16, tag="mu",
                                             name=f"mu_{qb}")
                            nc.scalar.dma_start(mu_q[:], muT[:][:, qb:qe, :, :])
                            rqs[qb], eqs[qb], mqs[qb] = rho_q, eps_q, mu_q
                        sps = {}
                        for qb, qe in batch:
                            for kg in range(qb, qe):
                                sp_b = spp.tile([P, KG, o_sh], bf16,
                                                tag=f"spb{kg % 8}",
                                                name=f"spb_{kg}")
                                nc.scalar.activation(sp_b[:],
                                                     rqs[qb][:, kg - qb], EXP)
                                sps[kg] = sp_b
                        if first:
                            nc.scalar.activation(b_sp[:], brho_t[:], EXP)
                        for qb, qe in batch:
                            for kg in range(qb, qe):
                                sp_l = wtmp.tile([P, KG, o_sh], bf16, tag="spl")
                                nc.scalar.activation(sp_l[:], sps[kg][:], LN,
                                                     bias=1.0)
                                pr_t = wtmp.tile([P, KG, o_sh], bf16, tag="pr")
                                nc.vector.tensor_mul(pr_t[:], sp_l[:],
                                                     eqs[qb][:, kg - qb])
                                w_t = wpool.tile([P, KG, o_sh], bf16,
                                                 tag=f"wT{kg}")
                                nc.vector.tensor_add(w_t[:], pr_t[:],
                                                     mqs[qb][:, kg - qb])
                                if _wrep == 0:
                                    wts.append(w_t)
                        if first:
                            nc.scalar.activation(b_sp[:], b_sp[:], LN,
                                                 bias=1.0)
                            nc.vector.tensor_mul(b_sb[:], b_sp[:], beps_t[:])
                            nc.vector.tensor_add(b_sb[:], b_sb[:], bmu_t[:])
                            first = False

            def close_group(ps, ms, n):
                o_t = opool.tile([P, NF], f32, tag="o")
                nc.vector.tensor_scalar_add(o_t[:], ps[:], b_sb[:, ms : ms + 1])
                # scalar HWDGE ring is idle once the weight stream ends
                nc.scalar.dma_start(
                    out3[:, ms, n * NF : (n + 1) * NF], o_t[:]
                )

            # ---- main loop: y^T[o, t] += w[o, i] * x[t, i]
            for _mrep in range(mrepeat):
                # Streaming prologue: NSTREAM chunks, k-outermost, so each
                # weight chunk is consumed on arrival (8 PSUM banks open).
                if _mrep > 0:
                    xs = [load_x(n) for n in range(NSTREAM)]
                pss = [[psum_pool.tile([P, NF], f32, tag="ps",
                                       name=f"ps_s{n}_{ms}")
                        for ms in range(MS)]
                       for n in range(NSTREAM)]
                for ko in range(KO):
                    w_sl = wts[ko // KG][:, ko % KG : ko % KG + 1, :]
                    for n in range(NSTREAM):
                        for ms in range(MS):
                            nc.tensor.matmul(
                                pss[n][ms][:],
                                w_sl[:, :, ms * P : (ms + 1) * P],
                                xs[n][ko // KOP][:, ko % KOP : ko % KOP + 1, :],
                                start=(ko == 0),
                                stop=(ko == KO - 1),
                            )
                for n in range(NSTREAM):
                    for ms in range(MS):
                        close_group(pss[n][ms], ms, n)

                # Steady state: weights resident; k-innermost (PE-dense).
                for n in range(NSTREAM, NT):
                    x_t = load_x(n)
                    for ms in range(MS):
                        ps = psum_pool.tile([P, NF], f32, tag="ps")
                        for ko in range(KO):
                            nc.tensor.matmul(
                                ps[:],
                                wts[ko // KG][:, ko % KG : ko % KG + 1,
                                              ms * P : (ms + 1) * P],
                                x_t[ko // KOP][:, ko % KOP : ko % KOP + 1, :],
                                start=(ko == 0),
                                stop=(ko == KO - 1),
                            )
                        close_group(ps, ms, n)

    nc.compile()
    return nc


def shard_inputs(x, weight_mu, weight_rho, bias_mu, bias_rho, eps_w, eps_b,
                 in_f=IN_F, o_sh=O_SH, tokens=TOKENS, ncores=NCORES):
    """Host-side layout + sharding: transpose to [in, out] / [in, tokens]."""
    bf16 = ml_dtypes.bfloat16
    MS = o_sh // P
    KG = 2
    KO = in_f // P
    xT_bf = np.ascontiguousarray(np.asarray(x, dtype=np.float32).astype(bf16).T)
    muT_bf = np.asarray(weight_mu, dtype=np.float32).astype(bf16)
    epsT_bf = np.asarray(eps_w, dtype=np.float32).astype(bf16)

    def pack_w(wt):
        # [in_f, o_sh] -> [P, KO//KG, KG, o_sh]; row r=(kg*KG+j)*128+p
        return np.ascontiguousarray(
            wt.reshape(KO // KG, KG, P, o_sh).transpose(2, 0, 1, 3))

    in_maps = []
    for c in range(ncores):
        sl = slice(c * o_sh, (c + 1) * o_sh)
        in_maps.append({
            "xT": xT_bf,
            "muT": pack_w(np.ascontiguousarray(muT_bf[sl, :].T)),
            "rhoT": pack_w(np.ascontiguousarray(
                np.asarray(weight_rho)[sl, :].T.astype(np.float16))),
            "epsT": pack_w(np.ascontiguousarray(epsT_bf[sl, :].T)),
            "bmu": np.ascontiguousarray(np.asarray(bias_mu)[sl].reshape(MS, P).T),
            "brho": np.ascontiguousarray(np.asarray(bias_rho)[sl].reshape(MS, P).T),
            "beps": np.ascontiguousarray(np.asarray(eps_b)[sl].reshape(MS, P).T),
        })
    return in_maps


_NC_CACHE = {}


def _get_nc():
    if "nc" not in _NC_CACHE:
        _NC_CACHE["nc"] = build_nc()
    return _NC_CACHE["nc"]


def kernel(x, weight_mu, weight_rho, bias_mu, bias_rho, eps_w, eps_b):
    from concourse import bass_utils

    nc = _get_nc()
    in_maps = shard_inputs(x, weight_mu, weight_rho, bias_mu, bias_rho, eps_w, eps_b)
    res = bass_utils.run_bass_kernel_spmd(nc, in_maps, core_ids=list(range(NCORES)))
    yT = np.concatenate([res.results[c]["out"] for c in range(NCORES)], axis=0)
    return np.ascontiguousarray(yT.T)
